# revision 6
# baseline (speedup 1.0000x reference)
"""Multi-head attention (B=2, N=2048, C=1024, H=16, D=64) on 8 TRN2 NeuronCores.

Sharding: data-parallel over the 2 batches x tensor-parallel over 4 head-groups
(4 heads each) -> 8 cores, no cross-core communication.

Per-core strategy (vs the f32r baseline):
  - bf16 activations/weights on the PE (same PE rate as f32r, half the DMA and
    SBUF), f32 PSUM accumulation throughout.
  - Cross-iteration software pipeline: the benchmark repeat loop is unrolled
    2x with double-buffered xT/qT/kT/v sets; iteration i's attention weaves
    iteration i+1's ENTIRE QKV projection (and its x DMA) into spare PE slots,
    so steady-state iterations have no QKV prefix stall and ACT (exp, the
    critical engine at ~147us busy) stays fed.
  - Attention in m-tile-pair groups g: per head, two K=64 S matmuls (auto
    row-tiled (0,0)/(64,0)) fill a [128,1024] 2-bank psum; one ACT exp per
    head per group straight out of PSUM -> bf16 ee; PV with the ones-column
    denominator trick (M=65).  S psums single-buffered per head (sA/sB) form
    a 4-bank ping-pong that hides all semaphore latencies; PV lags 2 groups
    so po-bank handoff (DVE osb copy) never blocks the S->exp chain.
  - Output tail (PE transpose + reciprocal + scale + DMA) deferred into the
    next block's stream, pot borrowing the weave psum bank.
"""

import os

import numpy as np

import concourse.bass as bass
import concourse.tile as tile
from concourse import bacc, mybir
from concourse.bass_utils import run_bass_kernel_spmd
from concourse.masks import make_identity

f32 = mybir.dt.float32
bf16 = mybir.dt.bfloat16
AF = mybir.ActivationFunctionType

B, N_TOK, C = 2, 2048, 1024
H, HD = 16, 64
SCALE = HD ** -0.5
NH = 4             # heads per core
NP = 2             # head pairs per core
GC = H // NH       # head groups (cores per batch)
CC = C // 128      # channel tiles (8)
TT = N_TOK // 128  # token tiles (16)
NB = N_TOK // 512  # n-blocks (4)
NG = TT // 2       # m-tile pair groups per n-block (8)
NGT = NP * NB * NG  # total attention groups (64)
W_COLS = NH * HD          # 256
W_COLS_V = NH * (HD + 1)  # 260: v padded with a ones column per head


def _build(repeats=1):
    # "tiled": K=64 S matmuls, auto row-tiled (0,0)/(64,0) pairs (concurrent
    # if HW cooperates, but S<->QKV/PV alternation switches tiling mode).
    # "padded": per-head zero-padded K=128 q/k tiles; serial full-array S,
    # no tiling-mode switches.
    s_padded = False
    nc = bacc.Bacc("TRN2", target_bir_lowering=False, debug=False,
                   enable_asserts=False, num_devices=8)

    xT_d = nc.dram_tensor("xt", [C, N_TOK], bf16, kind="ExternalInput")
    wq_d = nc.dram_tensor("wq", [128, CC, W_COLS], bf16, kind="ExternalInput")
    wk_d = nc.dram_tensor("wk", [128, CC, W_COLS], bf16, kind="ExternalInput")
    wv_d = nc.dram_tensor("wv", [128, CC, W_COLS_V], bf16, kind="ExternalInput")
    bq_d = nc.dram_tensor("bq", [128, NP], f32, kind="ExternalInput")
    bk_d = nc.dram_tensor("bk", [128, NP], f32, kind="ExternalInput")
    bv_d = nc.dram_tensor("bv", [128, W_COLS_V], f32, kind="ExternalInput")
    out_d = nc.dram_tensor("out", [N_TOK, W_COLS], f32, kind="ExternalOutput")

    with tile.TileContext(nc) as tc:
        with (
            tc.tile_pool(name="consts", bufs=1) as consts,
            tc.tile_pool(name="weights", bufs=1) as wpool,
            tc.tile_pool(name="qk", bufs=1) as qkpool,
            tc.tile_pool(name="vpool", bufs=1) as vpool,
            tc.tile_pool(name="xTp", bufs=1) as xTpool,
        ):
            bq_s = consts.tile([128, NP], f32, tag="bq")
            bk_s = consts.tile([128, NP], f32, tag="bk")
            bv_s = consts.tile([128, W_COLS_V], f32, tag="bv")
            wq_s = wpool.tile([128, CC, W_COLS], bf16, tag="wq")
            wk_s = wpool.tile([128, CC, W_COLS], bf16, tag="wk")
            wv_s = wpool.tile([128, CC, W_COLS_V], bf16, tag="wv")
            # double-buffered activation sets (cross-iteration pipeline);
            # padded mode: one tile per head (other 64 partitions zero)
            nqk = NH if s_padded else NP
            qTs = [[qkpool.tile([128, N_TOK], bf16, tag=f"qT{p}_{s}",
                                name=f"qT{p}_{s}") for p in range(nqk)]
                   for s in range(2)]
            kTs = [[qkpool.tile([128, N_TOK], bf16, tag=f"kT{p}_{s}",
                                name=f"kT{p}_{s}") for p in range(nqk)]
                   for s in range(2)]
            vSs = [[vpool.tile([128, W_COLS_V], bf16, tag=f"vS{mt}_{s}",
                               name=f"vS{mt}_{s}") for mt in range(TT)]
                   for s in range(2)]
            xTs = [[xTpool.tile([128, N_TOK], bf16, tag=f"xT{cc}_{s}",
                                name=f"xT{cc}_{s}") for cc in range(CC)]
                   for s in range(2)]

            with (
                tc.tile_pool(name="psum", bufs=1, space="PSUM") as psum,
                tc.tile_pool(name="epool", bufs=4) as epool,
                tc.tile_pool(name="opool", bufs=2) as opool,
            ):
                def dma_weights():
                    nc.sync.dma_start(out=bq_s[:], in_=bq_d.ap())
                    nc.sync.dma_start(out=bk_s[:], in_=bk_d.ap())
                    nc.sync.dma_start(out=bv_s[:], in_=bv_d.ap())
                    nc.sync.dma_start(out=wq_s[:], in_=wq_d.ap())
                    nc.scalar.dma_start(out=wk_s[:], in_=wk_d.ap())
                    nc.scalar.dma_start(out=wv_s[:], in_=wv_d.ap())

                def dma_x(s):
                    for cc in range(CC):
                        eng = nc.sync if cc % 2 == 0 else nc.scalar
                        eng.dma_start(
                            out=xTs[s][cc][:],
                            in_=xT_d.ap()[cc * 128:(cc + 1) * 128, :],
                        )

                def group_steps(w_s, dst, b_s, pair, tth, s):
                    # one q-or-k projection group for set s: two 512-token
                    # blocks, each a single-bank psum (tag pw, 2 slots)
                    # accumulated over cc; yields its PE cost (ns) every
                    # couple of chunks so the pacer can meter it into
                    # attention hook slots
                    dts = (qTs if dst == "q" else kTs)[s]
                    for t in range(2):
                        ttb = tth * 2 + t
                        blk = slice(ttb * 512, (ttb + 1) * 512)
                        psg = psum.tile([128, 512], f32, tag="pw", bufs=2,
                                        name=f"g{pair}{tth}{t}_{dst}_{s}")
                        for cc in range(CC):
                            nc.tensor.matmul(
                                psg[:],
                                w_s[:, cc, pair * 128:(pair + 1) * 128],
                                xTs[s][cc][:, blk],
                                start=(cc == 0), stop=(cc == CC - 1),
                            )
                            if cc % 2 == 1 and cc < CC - 1:
                                yield 427
                        if s_padded:
                            for h in range(2):
                                rows = slice(h * 64, h * 64 + 64)
                                nc.vector.tensor_scalar_add(
                                    dts[2 * pair + h][rows, blk],
                                    psg[rows, :],
                                    b_s[rows, pair:pair + 1],
                                )
                        else:
                            nc.vector.tensor_scalar_add(
                                dts[pair][:, blk], psg[:],
                                b_s[:, pair:pair + 1],
                            )
                        yield 477

                def v_tile(mt, s):
                    # one v m-tile for set s: single-bank psum over cc, then
                    # bias-add (+ones column) with bf16 convert
                    vps = psum.tile([128, 512], f32, tag="pw", bufs=2,
                                    name=f"vps{mt}_{s}")
                    for cc in range(CC):
                        nc.tensor.matmul(
                            vps[:, 0:W_COLS_V],
                            xTs[s][cc][:, mt * 128:(mt + 1) * 128],
                            wv_s[:, cc, :],
                            start=(cc == 0), stop=(cc == CC - 1),
                        )
                    nc.vector.tensor_add(vSs[s][mt][:], vps[:, 0:W_COLS_V],
                                         bv_s[:])

                def weave_units(s):
                    # next iteration's ENTIRE QKV, ordered so early units
                    # only need early xT chunks (DMA still in flight);
                    # yields the PE cost (ns) of each unit for the pacer
                    yield from group_steps(wk_s, "k", bk_s, 0, 0, s)
                    yield from group_steps(wk_s, "k", bk_s, 0, 1, s)
                    yield from group_steps(wq_s, "q", bq_s, 0, 0, s)
                    for mt in range(TT):
                        v_tile(mt, s)
                        yield 866
                    yield from group_steps(wq_s, "q", bq_s, 0, 1, s)
                    yield from group_steps(wk_s, "k", bk_s, 1, 0, s)
                    yield from group_steps(wk_s, "k", bk_s, 1, 1, s)
                    yield from group_steps(wq_s, "q", bq_s, 1, 0, s)
                    yield from group_steps(wq_s, "q", bq_s, 1, 1, s)

                def qkv_direct(s):
                    # prologue: set-s QKV with no attention to weave into
                    # (one-time cost, amortized out by the repeat loop)
                    for _ in weave_units(s):
                        pass

                def attn_phase(s):
                    """Attention over set s; weaves set s^1 QKV + x DMA."""
                    sn = 1 - s
                    dma_x(sn)
                    gen = weave_units(sn)
                    qTp, kTp, vSt = qTs[s], kTs[s], vSs[s]

                    # meter the weave into hook slots at a uniform ns rate so
                    # no slot's PE work spikes above the ACT slot time
                    pace = {"woven": 0.0, "slots": 0, "done": False}
                    rate = 42000.0 / (2 * (NGT + 2))

                    def hook():
                        pace["slots"] += 1
                        budget = (pace["slots"] * rate if rate < 1e8
                                  else pace["woven"] + 1)
                        while not pace["done"] and pace["woven"] < budget:
                            c = next(gen, None)
                            if c is None:
                                pace["done"] = True
                            else:
                                pace["woven"] += c
                            if rate >= 1e8:
                                break

                    po = {}          # (pair, nb) live po tiles
                    ees = {}         # gi -> (eeA, eeB)
                    tails = []

                    def emit_S_exp(gi):
                        pair, r = divmod(gi, NB * NG)
                        nb, g = divmod(r, NG)
                        nq = nb * 512
                        sA = psum.tile([128, 1024], f32, tag="sA",
                                       name=f"sA_{s}_{gi}")
                        sB = psum.tile([128, 1024], f32, tag="sB",
                                       name=f"sB_{s}_{gi}")
                        for ko in range(2):
                            mt = 2 * g + ko
                            mts = slice(mt * 128, (mt + 1) * 128)
                            if s_padded:
                                nc.tensor.matmul(
                                    sA[:, ko * 512:(ko + 1) * 512],
                                    kTp[2 * pair][:, mts],
                                    qTp[2 * pair][:, nq:nq + 512],
                                    start=True, stop=True,
                                )
                                nc.tensor.matmul(
                                    sB[:, ko * 512:(ko + 1) * 512],
                                    kTp[2 * pair + 1][:, mts],
                                    qTp[2 * pair + 1][:, nq:nq + 512],
                                    start=True, stop=True,
                                )
                            else:
                                nc.tensor.matmul(
                                    sA[:, ko * 512:(ko + 1) * 512],
                                    kTp[pair][0:64, mts],
                                    qTp[pair][0:64, nq:nq + 512],
                                    start=True, stop=True,
                                )
                                nc.tensor.matmul(
                                    sB[:, ko * 512:(ko + 1) * 512],
                                    kTp[pair][64:128, mts],
                                    qTp[pair][64:128, nq:nq + 512],
                                    start=True, stop=True,
                                )
                        eeA = epool.tile([128, 1024], bf16, tag="eeA")
                        eeB = epool.tile([128, 1024], bf16, tag="eeB")
                        nc.scalar.activation(eeA[:], sA[:], AF.Exp, scale=SCALE)
                        nc.scalar.activation(eeB[:], sB[:], AF.Exp, scale=SCALE)
                        ees[gi] = (eeA, eeB)

                    def emit_PV(gi):
                        # ee-stationary orientation: out[n-chunk, 65] =
                        # ee[m, n-chunk].T @ v_aug[m, 65].  Full 128-partition
                        # output halves PE columns vs the v-stationary form
                        # (out was [65, 512]), LDW (FWL bf16, 64cyc) hides
                        # under the 65-col matmuls, and the result is already
                        # token-major so no output transpose is needed.
                        pair, r = divmod(gi, NB * NG)
                        nb, g = divmod(r, NG)
                        if g == 0:
                            po[(pair, nb)] = (
                                psum.tile([128, 260], f32, tag="poA",
                                          name=f"poA_{s}_{pair}_{nb}"),
                                psum.tile([128, 260], f32, tag="poB",
                                          name=f"poB_{s}_{pair}_{nb}"),
                            )
                        po_A, po_B = po[(pair, nb)]
                        eeA, eeB = ees.pop(gi)
                        for ko in range(2):
                            mt = 2 * g + ko
                            for head, p, ee in ((2 * pair, po_A, eeA),
                                                (2 * pair + 1, po_B, eeB)):
                                vs = vSt[mt][:, (head % NH) * 65:
                                             (head % NH) * 65 + 65]
                                for ncb in range(4):
                                    # one accumulation group per po BANK:
                                    # start marks the whole 2KB zero-region
                                    # pending-zero, so only the bank's first
                                    # matmul may start (later regions
                                    # overwrite via pending-zero bytes)
                                    nc.tensor.matmul(
                                        p[:, ncb * 65:(ncb + 1) * 65],
                                        ee[:, ko * 512 + ncb * 128:
                                           ko * 512 + (ncb + 1) * 128],
                                        vs,
                                        start=(g == 0 and ko == 0
                                               and ncb == 0),
                                        stop=(g == NG - 1 and ko == 1
                                              and ncb == 3),
                                    )

                    def emit_tail_start(gi):
                        # PV for block (pair, nb) just finished: evacuate po
                        # on DVE now; heavy tail deferred into the next
                        # block's stream.  po is already token-major
                        # [128, 4(ncb), 65] with the denominator in col 64.
                        pair, r = divmod(gi, NB * NG)
                        nb, g = divmod(r, NG)
                        po_A, po_B = po.pop((pair, nb))
                        nq = nb * 512
                        osbs = []
                        for head, p in ((2 * pair, po_A), (2 * pair + 1, po_B)):
                            osb = opool.tile([128, 260], f32, tag="osb")
                            nc.vector.tensor_copy(osb[:], p[:])
                            osbs.append((head, osb))

                        rc = opool.tile([128, 8], f32, tag="rc")
                        fo = opool.tile([128, 8, HD], f32, tag="fo")

                        def head_tail(i, head, osb):
                            # fo is j-major so both heads' 64-col blocks sit
                            # adjacent and one 512B-per-line DMA covers the
                            # whole pair.
                            ov = osb[:].rearrange("p (j c) -> p j c", j=4)
                            nc.vector.reciprocal(rc[:, 4 * i:4 * i + 4],
                                                 ov[:, :, 64])
                            for j in range(4):
                                nc.vector.tensor_scalar_mul(
                                    fo[:, 2 * j + i, :],
                                    ov[:, j, 0:HD],
                                    rc[:, 4 * i + j:4 * i + j + 1],
                                )
                            if i == 1:
                                nc.sync.dma_start(
                                    out=out_d.ap()[nq:nq + 512,
                                                   pair * 128:
                                                   (pair + 1) * 128]
                                    .rearrange("(j p) (h d) -> p j h d",
                                               p=128, h=2),
                                    in_=fo[:].rearrange("p (j h) d -> p j h d",
                                                        h=2),
                                )
                        for i, (head, osb) in enumerate(osbs):
                            tails.append(
                                lambda i=i, head=head, osb=osb:
                                head_tail(i, head, osb))

                    for gi in range(NGT + 2):
                        if gi < NGT:
                            emit_S_exp(gi)
                        if gi >= 2:
                            emit_PV(gi - 2)
                            if (gi - 2) % NG == NG - 1:
                                emit_tail_start(gi - 2)
                        hook()
                        if gi % NG in (4, 5) and tails:
                            tails.pop(0)()
                        hook()
                    while tails:
                        tails.pop(0)()
                    for _ in gen:  # drain any unwoven units
                        pass

                def _prologue():
                    dma_weights()
                    dma_x(0)
                    warm = consts.tile([128, 1], f32, tag="warm")
                    nc.scalar.activation(warm[:], bq_s[:, 0:1], AF.Exp,
                                         scale=SCALE)
                    if s_padded:
                        # zero the unused partition half of each per-head
                        # q/k tile once; iterations only rewrite data rows
                        for st in range(2):
                            for h in range(NH):
                                zr = (slice(64, 128) if h % 2 == 0
                                      else slice(0, 64))
                                nc.vector.memset(qTs[st][h][zr, :], 0)
                                nc.vector.memset(kTs[st][h][zr, :], 0)
                    qkv_direct(0)

                _prologue()
                if repeats == 1:
                    attn_phase(0)
                elif os.environ.get("KERNEL_NOLOOP"):
                    # profiling: straight-line python unroll (TimelineSim
                    # cannot resolve HW-loop branches without an executor)
                    for _ in range(repeats // 2):
                        attn_phase(0)
                        attn_phase(1)
                else:
                    assert repeats % 2 == 0, "repeats must be 1 or even"
                    with tc.For_i(0, repeats // 2, 1):
                        attn_phase(0)
                        attn_phase(1)

    nc.compile()
    return nc


_NC = None


def _get_nc():
    global _NC
    if _NC is None:
        _NC = _build(repeats=int(os.environ.get("KERNEL_REPEATS", "1")))
    return _NC


def _in_maps(x, w_qkv, b_qkv):
    import ml_dtypes
    bf = ml_dtypes.bfloat16
    x = np.ascontiguousarray(x, dtype=np.float32)
    w_qkv = np.ascontiguousarray(w_qkv, dtype=np.float32)
    b_qkv = np.ascontiguousarray(b_qkv, dtype=np.float32)
    xts = [np.ascontiguousarray(x[b].T.astype(bf)) for b in range(B)]
    maps = []
    for core in range(8):
        b = core // GC
        g = core % GC
        cols = slice(g * W_COLS, (g + 1) * W_COLS)
        wq = w_qkv[:, 0 * C:1 * C][:, cols]
        wk = w_qkv[:, 1 * C:2 * C][:, cols]
        wv_raw = w_qkv[:, 2 * C:3 * C][:, cols]
        wv = np.zeros((C, W_COLS_V), dtype=np.float32)
        wv.reshape(C, NH, HD + 1)[:, :, 0:HD] = wv_raw.reshape(C, NH, HD)
        # [c, m] -> [p, cc, m] so the on-device DMA is fully contiguous
        wq = wq.reshape(CC, 128, W_COLS).transpose(1, 0, 2)
        wk = wk.reshape(CC, 128, W_COLS).transpose(1, 0, 2)
        wv = wv.reshape(CC, 128, W_COLS_V).transpose(1, 0, 2)
        bq = b_qkv[0 * C:1 * C][cols].reshape(NP, 128).T
        bk = b_qkv[1 * C:2 * C][cols].reshape(NP, 128).T
        bv_row = np.zeros((W_COLS_V,), dtype=np.float32)
        bv_row.reshape(NH, HD + 1)[:, 0:HD] = b_qkv[2 * C:3 * C][cols].reshape(NH, HD)
        bv_row.reshape(NH, HD + 1)[:, HD] = 1.0
        bv = np.broadcast_to(bv_row, (128, W_COLS_V))
        maps.append({
            "xt": xts[b],
            "wq": np.ascontiguousarray(wq.astype(bf)),
            "wk": np.ascontiguousarray(wk.astype(bf)),
            "wv": np.ascontiguousarray(wv.astype(bf)),
            "bq": np.ascontiguousarray(bq),
            "bk": np.ascontiguousarray(bk),
            "bv": np.ascontiguousarray(bv),
        })
    return maps


def kernel(x, w_qkv, b_qkv):
    nc = _get_nc()
    maps = _in_maps(x, w_qkv, b_qkv)
    res = run_bass_kernel_spmd(nc, maps, list(range(8)))
    y = np.empty((B, N_TOK, C), dtype=np.float32)
    for core in range(8):
        b = core // GC
        g = core % GC
        y[b, :, g * W_COLS:(g + 1) * W_COLS] = res.results[core]["out"]
    return y



# revision 7
# speedup vs baseline: 1.0961x; 1.0961x over previous
"""Multi-head attention (B=2, N=2048, C=1024, H=16, D=64) on 8 TRN2 NeuronCores.

Sharding: data-parallel over the 2 batches x tensor-parallel over 4 head-groups
(4 heads each) -> 8 cores, no cross-core communication.

Per-core strategy (vs the f32r baseline):
  - bf16 activations/weights on the PE (same PE rate as f32r, half the DMA and
    SBUF), f32 PSUM accumulation throughout.
  - Cross-iteration software pipeline: the benchmark repeat loop is unrolled
    2x with double-buffered xT/qT/kT/v sets; iteration i's attention weaves
    iteration i+1's ENTIRE QKV projection (and its x DMA) into spare PE slots,
    so steady-state iterations have no QKV prefix stall and ACT (exp, the
    critical engine at ~147us busy) stays fed.
  - Attention in m-tile-pair groups g: per head, two K=64 S matmuls (auto
    row-tiled (0,0)/(64,0)) fill a [128,1024] 2-bank psum; one ACT exp per
    head per group straight out of PSUM -> bf16 ee; PV with the ones-column
    denominator trick (M=65).  S psums single-buffered per head (sA/sB) form
    a 4-bank ping-pong that hides all semaphore latencies; PV lags 2 groups
    so po-bank handoff (DVE osb copy) never blocks the S->exp chain.
  - Output tail (PE transpose + reciprocal + scale + DMA) deferred into the
    next block's stream, pot borrowing the weave psum bank.
"""

import os

import numpy as np

import concourse.bass as bass
import concourse.tile as tile
from concourse import bacc, mybir
from concourse.bass_utils import run_bass_kernel_spmd
from concourse.masks import make_identity

f32 = mybir.dt.float32
bf16 = mybir.dt.bfloat16
AF = mybir.ActivationFunctionType

B, N_TOK, C = 2, 2048, 1024
H, HD = 16, 64
SCALE = HD ** -0.5
NH = 4             # heads per core
NP = 2             # head pairs per core
GC = H // NH       # head groups (cores per batch)
CC = C // 128      # channel tiles (8)
TT = N_TOK // 128  # token tiles (16)
NB = N_TOK // 512  # n-blocks (4)
NG = TT // 2       # m-tile pair groups per n-block (8)
NGT = NP * NB * NG  # total attention groups (64)
W_COLS = NH * HD          # 256
W_COLS_V = NH * (HD + 1)  # 260: v padded with a ones column per head


def _build(repeats=1):
    # "tiled": K=64 S matmuls, auto row-tiled (0,0)/(64,0) pairs (concurrent
    # if HW cooperates, but S<->QKV/PV alternation switches tiling mode).
    # "padded": per-head zero-padded K=128 q/k tiles; serial full-array S,
    # no tiling-mode switches.
    s_padded = False
    nc = bacc.Bacc("TRN2", target_bir_lowering=False, debug=False,
                   enable_asserts=False, num_devices=8)

    xT_d = nc.dram_tensor("xt", [C, N_TOK], bf16, kind="ExternalInput")
    wq_d = nc.dram_tensor("wq", [128, CC, W_COLS], bf16, kind="ExternalInput")
    wk_d = nc.dram_tensor("wk", [128, CC, W_COLS], bf16, kind="ExternalInput")
    wv_d = nc.dram_tensor("wv", [128, CC, W_COLS_V], bf16, kind="ExternalInput")
    bq_d = nc.dram_tensor("bq", [128, NP], f32, kind="ExternalInput")
    bk_d = nc.dram_tensor("bk", [128, NP], f32, kind="ExternalInput")
    bv_d = nc.dram_tensor("bv", [128, W_COLS_V], f32, kind="ExternalInput")
    out_d = nc.dram_tensor("out", [N_TOK, W_COLS], f32, kind="ExternalOutput")

    with tile.TileContext(nc) as tc:
        with (
            tc.tile_pool(name="consts", bufs=1) as consts,
            tc.tile_pool(name="weights", bufs=1) as wpool,
            tc.tile_pool(name="qk", bufs=1) as qkpool,
            tc.tile_pool(name="vpool", bufs=1) as vpool,
            tc.tile_pool(name="xTp", bufs=1) as xTpool,
        ):
            bq_s = consts.tile([128, NP], f32, tag="bq")
            bk_s = consts.tile([128, NP], f32, tag="bk")
            bv_s = consts.tile([128, W_COLS_V], f32, tag="bv")
            wq_s = wpool.tile([128, CC, W_COLS], bf16, tag="wq")
            wk_s = wpool.tile([128, CC, W_COLS], bf16, tag="wk")
            wv_s = wpool.tile([128, CC, W_COLS_V], bf16, tag="wv")
            # double-buffered activation sets (cross-iteration pipeline);
            # padded mode: one tile per head (other 64 partitions zero)
            nqk = NH if s_padded else NP
            qTs = [[qkpool.tile([128, N_TOK], bf16, tag=f"qT{p}_{s}",
                                name=f"qT{p}_{s}") for p in range(nqk)]
                   for s in range(2)]
            kTs = [[qkpool.tile([128, N_TOK], bf16, tag=f"kT{p}_{s}",
                                name=f"kT{p}_{s}") for p in range(nqk)]
                   for s in range(2)]
            vSs = [[vpool.tile([128, W_COLS_V], bf16, tag=f"vS{mt}_{s}",
                               name=f"vS{mt}_{s}") for mt in range(TT)]
                   for s in range(2)]
            xTs = [[xTpool.tile([128, N_TOK], bf16, tag=f"xT{cc}_{s}",
                                name=f"xT{cc}_{s}") for cc in range(CC)]
                   for s in range(2)]

            with (
                tc.tile_pool(name="psum", bufs=1, space="PSUM") as psum,
                tc.tile_pool(name="epool", bufs=4) as epool,
                tc.tile_pool(name="opool", bufs=2) as opool,
            ):
                def dma_weights():
                    nc.sync.dma_start(out=bq_s[:], in_=bq_d.ap())
                    nc.sync.dma_start(out=bk_s[:], in_=bk_d.ap())
                    nc.sync.dma_start(out=bv_s[:], in_=bv_d.ap())
                    nc.sync.dma_start(out=wq_s[:], in_=wq_d.ap())
                    nc.scalar.dma_start(out=wk_s[:], in_=wk_d.ap())
                    nc.scalar.dma_start(out=wv_s[:], in_=wv_d.ap())

                def dma_x(s):
                    for cc in range(CC):
                        eng = nc.sync if cc % 2 == 0 else nc.scalar
                        eng.dma_start(
                            out=xTs[s][cc][:],
                            in_=xT_d.ap()[cc * 128:(cc + 1) * 128, :],
                        )

                def group_steps(w_s, dst, b_s, pair, tth, s):
                    # one q-or-k projection group for set s: two 512-token
                    # blocks, each a single-bank psum (tag pw, 2 slots)
                    # accumulated over cc; yields its PE cost (ns) every
                    # couple of chunks so the pacer can meter it into
                    # attention hook slots
                    dts = (qTs if dst == "q" else kTs)[s]
                    for t in range(2):
                        ttb = tth * 2 + t
                        blk = slice(ttb * 512, (ttb + 1) * 512)
                        psg = psum.tile([128, 512], f32, tag="pw", bufs=2,
                                        name=f"g{pair}{tth}{t}_{dst}_{s}")
                        for cc in range(CC):
                            nc.tensor.matmul(
                                psg[:],
                                w_s[:, cc, pair * 128:(pair + 1) * 128],
                                xTs[s][cc][:, blk],
                                start=(cc == 0), stop=(cc == CC - 1),
                            )
                            if cc % 2 == 1 and cc < CC - 1:
                                yield 427
                        if s_padded:
                            for h in range(2):
                                rows = slice(h * 64, h * 64 + 64)
                                nc.vector.tensor_scalar_add(
                                    dts[2 * pair + h][rows, blk],
                                    psg[rows, :],
                                    b_s[rows, pair:pair + 1],
                                )
                        else:
                            nc.vector.tensor_scalar_add(
                                dts[pair][:, blk], psg[:],
                                b_s[:, pair:pair + 1],
                            )
                        yield 477

                def v_tile(mt, s):
                    # one v m-tile for set s: single-bank psum over cc, then
                    # bias-add (+ones column) with bf16 convert
                    vps = psum.tile([128, 512], f32, tag="pw", bufs=2,
                                    name=f"vps{mt}_{s}")
                    for cc in range(CC):
                        nc.tensor.matmul(
                            vps[:, 0:W_COLS_V],
                            xTs[s][cc][:, mt * 128:(mt + 1) * 128],
                            wv_s[:, cc, :],
                            start=(cc == 0), stop=(cc == CC - 1),
                        )
                    nc.vector.tensor_add(vSs[s][mt][:], vps[:, 0:W_COLS_V],
                                         bv_s[:])

                def weave_units(s):
                    # next iteration's ENTIRE QKV, ordered so early units
                    # only need early xT chunks (DMA still in flight);
                    # yields the PE cost (ns) of each unit for the pacer
                    yield from group_steps(wk_s, "k", bk_s, 0, 0, s)
                    yield from group_steps(wk_s, "k", bk_s, 0, 1, s)
                    yield from group_steps(wq_s, "q", bq_s, 0, 0, s)
                    for mt in range(TT):
                        v_tile(mt, s)
                        yield 866
                    yield from group_steps(wq_s, "q", bq_s, 0, 1, s)
                    yield from group_steps(wk_s, "k", bk_s, 1, 0, s)
                    yield from group_steps(wk_s, "k", bk_s, 1, 1, s)
                    yield from group_steps(wq_s, "q", bq_s, 1, 0, s)
                    yield from group_steps(wq_s, "q", bq_s, 1, 1, s)

                def qkv_direct(s):
                    # prologue: set-s QKV with no attention to weave into
                    # (one-time cost, amortized out by the repeat loop)
                    for _ in weave_units(s):
                        pass

                def attn_phase(s):
                    """Attention over set s; weaves set s^1 QKV + x DMA."""
                    sn = 1 - s
                    dma_x(sn)
                    gen = weave_units(sn)
                    qTp, kTp, vSt = qTs[s], kTs[s], vSs[s]

                    # meter the weave into hook slots at a uniform ns rate so
                    # no slot's PE work spikes above the ACT slot time
                    pace = {"woven": 0.0, "slots": 0, "done": False}
                    rate = 42000.0 / (2 * (NGT + 2))

                    def hook():
                        pace["slots"] += 1
                        budget = (pace["slots"] * rate if rate < 1e8
                                  else pace["woven"] + 1)
                        while not pace["done"] and pace["woven"] < budget:
                            c = next(gen, None)
                            if c is None:
                                pace["done"] = True
                            else:
                                pace["woven"] += c
                            if rate >= 1e8:
                                break

                    po = {}          # (pair, nb) live po tiles
                    ees = {}         # gi -> (eeA, eeB)
                    tails = []

                    def emit_S_exp(gi):
                        pair, r = divmod(gi, NB * NG)
                        nb, g = divmod(r, NG)
                        nq = nb * 512
                        sA = psum.tile([128, 1024], f32, tag="sA",
                                       name=f"sA_{s}_{gi}")
                        sB = psum.tile([128, 1024], f32, tag="sB",
                                       name=f"sB_{s}_{gi}")
                        # emit ALL of sA's matmuls before sB's: the PE queue
                        # is in-order, so interleaving makes sA's second MM
                        # transitively wait on exp-B (HW chain probe: +47us
                        # of ACT stall).  A-then-B lets sA(i+1) fill during
                        # exp(i)-B's window and ACT never starves.
                        if s_padded:
                            halves = ((sA, qTp[2 * pair], kTp[2 * pair],
                                       slice(0, 128)),
                                      (sB, qTp[2 * pair + 1],
                                       kTp[2 * pair + 1], slice(0, 128)))
                        else:
                            halves = ((sA, qTp[pair], kTp[pair],
                                       slice(0, 64)),
                                      (sB, qTp[pair], kTp[pair],
                                       slice(64, 128)))
                        for sd, qT, kT, rows in halves:
                            for ko in range(2):
                                mt = 2 * g + ko
                                mts = slice(mt * 128, (mt + 1) * 128)
                                nc.tensor.matmul(
                                    sd[:, ko * 512:(ko + 1) * 512],
                                    kT[rows, mts],
                                    qT[rows, nq:nq + 512],
                                    start=True, stop=True,
                                )
                        eeA = epool.tile([128, 1024], bf16, tag="eeA")
                        eeB = epool.tile([128, 1024], bf16, tag="eeB")
                        nc.scalar.activation(eeA[:], sA[:], AF.Exp, scale=SCALE)
                        nc.scalar.activation(eeB[:], sB[:], AF.Exp, scale=SCALE)
                        ees[gi] = (eeA, eeB)

                    def emit_PV(gi):
                        # ee-stationary orientation: out[n-chunk, 65] =
                        # ee[m, n-chunk].T @ v_aug[m, 65].  Full 128-partition
                        # output halves PE columns vs the v-stationary form
                        # (out was [65, 512]), LDW (FWL bf16, 64cyc) hides
                        # under the 65-col matmuls, and the result is already
                        # token-major so no output transpose is needed.
                        pair, r = divmod(gi, NB * NG)
                        nb, g = divmod(r, NG)
                        if g == 0:
                            po[(pair, nb)] = (
                                psum.tile([128, 260], f32, tag="poA",
                                          name=f"poA_{s}_{pair}_{nb}"),
                                psum.tile([128, 260], f32, tag="poB",
                                          name=f"poB_{s}_{pair}_{nb}"),
                            )
                        po_A, po_B = po[(pair, nb)]
                        eeA, eeB = ees.pop(gi)
                        for ko in range(2):
                            mt = 2 * g + ko
                            for head, p, ee in ((2 * pair, po_A, eeA),
                                                (2 * pair + 1, po_B, eeB)):
                                vs = vSt[mt][:, (head % NH) * 65:
                                             (head % NH) * 65 + 65]
                                for ncb in range(4):
                                    # one accumulation group per po BANK:
                                    # start marks the whole 2KB zero-region
                                    # pending-zero, so only the bank's first
                                    # matmul may start (later regions
                                    # overwrite via pending-zero bytes)
                                    nc.tensor.matmul(
                                        p[:, ncb * 65:(ncb + 1) * 65],
                                        ee[:, ko * 512 + ncb * 128:
                                           ko * 512 + (ncb + 1) * 128],
                                        vs,
                                        start=(g == 0 and ko == 0
                                               and ncb == 0),
                                        stop=(g == NG - 1 and ko == 1
                                              and ncb == 3),
                                    )

                    def emit_tail_start(gi):
                        # PV for block (pair, nb) just finished: evacuate po
                        # on DVE now; heavy tail deferred into the next
                        # block's stream.  po is already token-major
                        # [128, 4(ncb), 65] with the denominator in col 64.
                        pair, r = divmod(gi, NB * NG)
                        nb, g = divmod(r, NG)
                        po_A, po_B = po.pop((pair, nb))
                        nq = nb * 512
                        osbs = []
                        for head, p in ((2 * pair, po_A), (2 * pair + 1, po_B)):
                            osb = opool.tile([128, 260], f32, tag="osb")
                            nc.vector.tensor_copy(osb[:], p[:])
                            osbs.append((head, osb))

                        rc = opool.tile([128, 8], f32, tag="rc")
                        fo = opool.tile([128, 8, HD], f32, tag="fo")

                        def head_tail(i, head, osb):
                            # fo is j-major so both heads' 64-col blocks sit
                            # adjacent and one 512B-per-line DMA covers the
                            # whole pair.
                            ov = osb[:].rearrange("p (j c) -> p j c", j=4)
                            nc.vector.reciprocal(rc[:, 4 * i:4 * i + 4],
                                                 ov[:, :, 64])
                            for j in range(4):
                                nc.vector.tensor_scalar_mul(
                                    fo[:, 2 * j + i, :],
                                    ov[:, j, 0:HD],
                                    rc[:, 4 * i + j:4 * i + j + 1],
                                )
                            if i == 1:
                                nc.sync.dma_start(
                                    out=out_d.ap()[nq:nq + 512,
                                                   pair * 128:
                                                   (pair + 1) * 128]
                                    .rearrange("(j p) (h d) -> p j h d",
                                               p=128, h=2),
                                    in_=fo[:].rearrange("p (j h) d -> p j h d",
                                                        h=2),
                                )
                        for i, (head, osb) in enumerate(osbs):
                            tails.append(
                                lambda i=i, head=head, osb=osb:
                                head_tail(i, head, osb))

                    for gi in range(NGT + 2):
                        if gi < NGT:
                            emit_S_exp(gi)
                        if gi >= 2:
                            emit_PV(gi - 2)
                            if (gi - 2) % NG == NG - 1:
                                emit_tail_start(gi - 2)
                        hook()
                        if gi % NG in (4, 5) and tails:
                            tails.pop(0)()
                        hook()
                    while tails:
                        tails.pop(0)()
                    for _ in gen:  # drain any unwoven units
                        pass

                def _prologue():
                    dma_weights()
                    dma_x(0)
                    warm = consts.tile([128, 1], f32, tag="warm")
                    nc.scalar.activation(warm[:], bq_s[:, 0:1], AF.Exp,
                                         scale=SCALE)
                    if s_padded:
                        # zero the unused partition half of each per-head
                        # q/k tile once; iterations only rewrite data rows
                        for st in range(2):
                            for h in range(NH):
                                zr = (slice(64, 128) if h % 2 == 0
                                      else slice(0, 64))
                                nc.vector.memset(qTs[st][h][zr, :], 0)
                                nc.vector.memset(kTs[st][h][zr, :], 0)
                    qkv_direct(0)

                _prologue()
                if repeats == 1:
                    attn_phase(0)
                elif os.environ.get("KERNEL_NOLOOP"):
                    # profiling: straight-line python unroll (TimelineSim
                    # cannot resolve HW-loop branches without an executor)
                    for _ in range(repeats // 2):
                        attn_phase(0)
                        attn_phase(1)
                else:
                    assert repeats % 2 == 0, "repeats must be 1 or even"
                    with tc.For_i(0, repeats // 2, 1):
                        attn_phase(0)
                        attn_phase(1)

    nc.compile()
    return nc


_NC = None


def _get_nc():
    global _NC
    if _NC is None:
        _NC = _build(repeats=int(os.environ.get("KERNEL_REPEATS", "1")))
    return _NC


def _in_maps(x, w_qkv, b_qkv):
    import ml_dtypes
    bf = ml_dtypes.bfloat16
    x = np.ascontiguousarray(x, dtype=np.float32)
    w_qkv = np.ascontiguousarray(w_qkv, dtype=np.float32)
    b_qkv = np.ascontiguousarray(b_qkv, dtype=np.float32)
    xts = [np.ascontiguousarray(x[b].T.astype(bf)) for b in range(B)]
    maps = []
    for core in range(8):
        b = core // GC
        g = core % GC
        cols = slice(g * W_COLS, (g + 1) * W_COLS)
        wq = w_qkv[:, 0 * C:1 * C][:, cols]
        wk = w_qkv[:, 1 * C:2 * C][:, cols]
        wv_raw = w_qkv[:, 2 * C:3 * C][:, cols]
        wv = np.zeros((C, W_COLS_V), dtype=np.float32)
        wv.reshape(C, NH, HD + 1)[:, :, 0:HD] = wv_raw.reshape(C, NH, HD)
        # [c, m] -> [p, cc, m] so the on-device DMA is fully contiguous
        wq = wq.reshape(CC, 128, W_COLS).transpose(1, 0, 2)
        wk = wk.reshape(CC, 128, W_COLS).transpose(1, 0, 2)
        wv = wv.reshape(CC, 128, W_COLS_V).transpose(1, 0, 2)
        bq = b_qkv[0 * C:1 * C][cols].reshape(NP, 128).T
        bk = b_qkv[1 * C:2 * C][cols].reshape(NP, 128).T
        bv_row = np.zeros((W_COLS_V,), dtype=np.float32)
        bv_row.reshape(NH, HD + 1)[:, 0:HD] = b_qkv[2 * C:3 * C][cols].reshape(NH, HD)
        bv_row.reshape(NH, HD + 1)[:, HD] = 1.0
        bv = np.broadcast_to(bv_row, (128, W_COLS_V))
        maps.append({
            "xt": xts[b],
            "wq": np.ascontiguousarray(wq.astype(bf)),
            "wk": np.ascontiguousarray(wk.astype(bf)),
            "wv": np.ascontiguousarray(wv.astype(bf)),
            "bq": np.ascontiguousarray(bq),
            "bk": np.ascontiguousarray(bk),
            "bv": np.ascontiguousarray(bv),
        })
    return maps


def kernel(x, w_qkv, b_qkv):
    nc = _get_nc()
    maps = _in_maps(x, w_qkv, b_qkv)
    res = run_bass_kernel_spmd(nc, maps, list(range(8)))
    y = np.empty((B, N_TOK, C), dtype=np.float32)
    for core in range(8):
        b = core // GC
        g = core % GC
        y[b, :, g * W_COLS:(g + 1) * W_COLS] = res.results[core]["out"]
    return y

